# revision 24
# baseline (speedup 1.0000x reference)
"""Trainium2 Bass kernel for nn_MultiHeadLocalAttention_1683627180144.

Full-input contract: kernel(**inputs) takes the complete unsharded inputs and
returns the complete output.  Internally the work is sharded 8 ways
(batch x sequence-half, with a one-window halo) and run SPMD on 8 NeuronCores
via run_bass_kernel_spmd.  Per-core differences (positions for RoPE, edge
masks) are carried entirely by the per-core input data so every core runs the
same program.

Device-side dataflow (per core, 2176 tokens = 128-token halo + 2048 queries):
  1. LayerNorm (token-major) -> xn bf16; ln_w/ln_b are folded into the weights
     on the host, biases are applied on-device (general, usually zero).
  2. xn transposed to xnT [dim, tok] via DMA-xbar transpose.
  3. q^T,k^T = w_qk^T @ xn (dim-major out), v token-major, gates head-major.
  4. RoPE on q^T/k^T with host-precomputed cos/sin tables (q pre-scaled 1/8).
  5. Local attention per head in transposed orientation:
       sim^T[key,query] per key-window (PE) -> exp (ACT) -> mask-mul (DVE)
       -> out^T accumulated over key-windows with an appended ones-column in V
          so row 64 of the PSUM tile is the softmax denominator.
  6. scale = sigmoid-gate / denominator, broadcast across partitions via small
     DMAs, multiplied into out^T -> y^T slabs bf16.
  7. out = y^T.T @ w_out (token-major) -> f32 -> DRAM.
"""

import os
import sys

import numpy as np

for _p in ("/opt/trn_rl_repo", "/opt/pypackages"):
    if os.path.isdir(_p) and _p not in sys.path:
        sys.path.append(_p)

import ml_dtypes  # noqa: E402

import concourse.bass as bass  # noqa: E402
import concourse.bacc as bacc  # noqa: E402
import concourse.mybir as mybir  # noqa: E402
from concourse.tile import TileContext  # noqa: E402

BF16 = ml_dtypes.bfloat16

# Problem constants (hardcoded per spec).
B, N, DIM = 4, 4096, 1024
H, DH, WS = 16, 64, 128
NCORES = 8
P = 128
T = 2176          # tokens per shard incl halo window
NT = T // P       # 17 token windows
NQ = 2048         # query tokens per shard
KB = DIM // P     # 8 contraction chunks
QSCALE = DH ** -0.5

f32 = mybir.dt.float32
bf16 = mybir.dt.bfloat16

_PROGRAM_CACHE = {}


def _build_nc():
    """Build the per-core Bass program (same program on all 8 cores)."""
    nc = bacc.Bacc("TRN2")

    x_d = nc.declare_dram_parameter("x", [T, DIM], f32, isOutput=False)
    wqk_d = nc.declare_dram_parameter("wqk", [DIM, 2 * DIM], bf16, isOutput=False)
    wv_d = nc.declare_dram_parameter("wv", [DIM, DIM], bf16, isOutput=False)
    wg_d = nc.declare_dram_parameter("wg", [DIM, H], bf16, isOutput=False)
    wo_d = nc.declare_dram_parameter("wo", [DIM, DIM], bf16, isOutput=False)
    bqk_d = nc.declare_dram_parameter("bqk", [2 * DIM, 1], f32, isOutput=False)
    bv_d = nc.declare_dram_parameter("bv", [1, DIM], bf16, isOutput=False)
    bg_d = nc.declare_dram_parameter("bg", [H, 1], f32, isOutput=False)
    cos_d = nc.declare_dram_parameter("cos", [P, T], bf16, isOutput=False)
    sin_d = nc.declare_dram_parameter("sin", [P, T], bf16, isOutput=False)
    m0_d = nc.declare_dram_parameter("mask0", [P, 1024], bf16, isOutput=False)
    mr_d = nc.declare_dram_parameter("maskr", [P, 1024], bf16, isOutput=False)
    out_d = nc.declare_dram_parameter("out", [NQ, DIM], f32, isOutput=True)
    # internal DRAM bounce buffer for the per-query scale broadcast
    scr_d = nc.dram_tensor("scalescr", [H, 2, 1024], bf16)

    with TileContext(nc) as tc:
        from contextlib import ExitStack

        with ExitStack() as ctx:
            consts = ctx.enter_context(tc.tile_pool(name="consts", bufs=1))
            persist = ctx.enter_context(tc.tile_pool(name="persist", bufs=1))

            # Constants used across phases.
            mask0 = consts.tile([P, 1024], bf16, tag="mask0")
            maskr = consts.tile([P, 1024], bf16, tag="maskr")
            nc.gpsimd.dma_start(mask0, m0_d[:, :])
            nc.gpsimd.dma_start(maskr, mr_d[:, :])
            bg_t = consts.tile([H, 1], f32, tag="bg")
            nc.gpsimd.dma_start(bg_t, bg_d[:, :])
            eps_t = consts.tile([P, 1], f32, tag="eps")
            nc.vector.memset(eps_t, 1e-5)
            bqk_t = consts.tile([P, 16], f32, tag="bqk")
            nc.gpsimd.dma_start(bqk_t, bqk_d[:, :].rearrange("(m p) o -> p (m o)", p=P))
            bv_t = consts.tile([P, DIM], bf16, tag="bv")
            nc.gpsimd.dma_start(bv_t, bv_d[:, :].partition_broadcast(P)[:, 0, :])
            ident = consts.tile([P, P], bf16, tag="ident")
            from concourse.masks import make_identity
            make_identity(nc, ident)
            cos_t = consts.tile([P, T], bf16, tag="cos")
            sin_t = consts.tile([P, T], bf16, tag="sin")
            nc.gpsimd.dma_start(cos_t, cos_d[:, :])
            nc.gpsimd.dma_start(sin_t, sin_d[:, :])

            # Persistent state.
            qkT = [persist.tile([P, T], bf16, tag=f"qkT{i}", name=f"qkT{i}") for i in range(16)]
            vsl = [persist.tile([P, H * 65], bf16, tag=f"v{i}", name=f"v{i}") for i in range(NT)]
            gatesT = persist.tile([H, T], f32, tag="gatesT")

            # ---------------- Phase 1: LN, transposes, projections ----------
            with ExitStack() as ph1:
                p_x = ph1.enter_context(tc.tile_pool(name="p_x", bufs=2))
                p_xn = ph1.enter_context(tc.tile_pool(name="p_xn", bufs=3))
                p_st = ph1.enter_context(tc.tile_pool(name="p_st", bufs=4))
                p_xnT = ph1.enter_context(tc.tile_pool(name="p_xnT", bufs=1))
                p_w = ph1.enter_context(tc.tile_pool(name="p_w", bufs=2))
                p_wres = ph1.enter_context(tc.tile_pool(name="p_wres", bufs=1))
                p_rope = ph1.enter_context(tc.tile_pool(name="p_rope", bufs=2))
                p_ps1 = ph1.enter_context(
                    tc.tile_pool(name="p_ps1", bufs=4, space="PSUM")
                )
                p_ps1v = ph1.enter_context(
                    tc.tile_pool(name="p_ps1v", bufs=2, space="PSUM")
                )
                p_ps1g = ph1.enter_context(
                    tc.tile_pool(name="p_ps1g", bufs=2, space="PSUM")
                )

                xnT = [p_xnT.tile([P, T], bf16, tag=f"xnT{k}", name=f"xnT{k}") for k in range(KB)]

                # Resident v/gate weights.
                wv_sb = [p_wres.tile([P, DIM], bf16, tag=f"wv{k}", name=f"wv{k}") for k in range(KB)]
                wg_sb = [p_wres.tile([P, H], bf16, tag=f"wg{k}", name=f"wg{k}") for k in range(KB)]
                for k in range(KB):
                    nc.gpsimd.dma_start(wv_sb[k], wv_d[k * P : (k + 1) * P, :])
                    nc.gpsimd.dma_start(wg_sb[k], wg_d[k * P : (k + 1) * P, :])

                # 1a: LayerNorm + transpose per token tile.
                for mt in range(NT):
                    x_t = p_x.tile([P, DIM], f32, tag="x")
                    nc.gpsimd.dma_start(x_t, x_d[mt * P : (mt + 1) * P, :])
                    st = p_st.tile([P, 2, 6], f32, tag="st")
                    nc.vector.bn_stats(st[:, 0, :], x_t[:, 0:512])
                    nc.vector.bn_stats(st[:, 1, :], x_t[:, 512:1024])
                    mv = p_st.tile([P, 2], f32, tag="mv")
                    nc.vector.bn_aggr(mv, st)
                    rs = p_st.tile([P, 1], f32, tag="rs")
                    nc.scalar.activation(
                        rs, mv[:, 1:2], mybir.ActivationFunctionType.Sqrt,
                        bias=eps_t, scale=1.0,
                    )
                    nc.vector.reciprocal(rs, rs)
                    xn_t = p_xn.tile([P, DIM], bf16, tag="xn")
                    nc.vector.tensor_scalar(
                        out=xn_t, in0=x_t,
                        scalar1=mv[:, 0:1], scalar2=rs,
                        op0=mybir.AluOpType.subtract, op1=mybir.AluOpType.mult,
                    )
                    for k in range(KB):
                        nc.sync.dma_start_transpose(
                            xnT[k][:, mt * P : (mt + 1) * P],
                            xn_t[:, k * P : (k + 1) * P],
                        )

                nspans = [(i * 512, min((i + 1) * 512, T)) for i in range(5)]

                # 1b: q/k projection (dim-major output).
                for mqk in range(16):
                    w_t = p_w.tile([P, KB, P], bf16, tag="wqk")
                    nc.gpsimd.dma_start(
                        w_t,
                        wqk_d[:, mqk * P : (mqk + 1) * P].rearrange(
                            "(kb p) c -> p kb c", p=P
                        ),
                    )
                    for (n0, n1) in nspans:
                        ps = p_ps1.tile([P, 512], f32, tag="psqk")
                        for k in range(KB):
                            nc.tensor.matmul(
                                ps[:, : n1 - n0],
                                w_t[:, k, :],
                                xnT[k][:, n0:n1],
                                start=(k == 0),
                                stop=(k == KB - 1),
                            )
                        nc.scalar.activation(
                            qkT[mqk][:, n0:n1], ps[:, : n1 - n0],
                            mybir.ActivationFunctionType.Identity,
                            bias=bqk_t[:, mqk : mqk + 1],
                        )

                # 1c: v projection (token-major) into the slab with ones cols.
                for mt in range(NT):
                    nc.vector.memset(
                        vsl[mt].rearrange("p (h c) -> p h c", c=65)[:, :, 64:65], 1.0
                    )
                    for half in range(2):
                        ps = p_ps1v.tile([P, 512], f32, tag="psv")
                        for k in range(KB):
                            nc.tensor.matmul(
                                ps,
                                xnT[k][:, mt * P : (mt + 1) * P],
                                wv_sb[k][:, half * 512 : (half + 1) * 512],
                                start=(k == 0),
                                stop=(k == KB - 1),
                            )
                        # out = (psum * 1.0) + bias_v, strided into the slab.
                        nc.vector.scalar_tensor_tensor(
                            out=vsl[mt].rearrange("p (h c) -> p h c", c=65)[
                                :, half * 8 : (half + 1) * 8, 0:64
                            ],
                            in0=ps.rearrange("p (h c) -> p h c", c=64),
                            scalar=1.0,
                            in1=bv_t.rearrange("p (h c) -> p h c", c=64)[
                                :, half * 8 : (half + 1) * 8, :
                            ],
                            op0=mybir.AluOpType.mult,
                            op1=mybir.AluOpType.add,
                        )

                # 1d: gates (head-major), sigmoid fused with bias.
                for (n0, n1) in nspans:
                    ps = p_ps1g.tile([H, 512], f32, tag="psg")
                    for k in range(KB):
                        nc.tensor.matmul(
                            ps[:, : n1 - n0],
                            wg_sb[k],
                            xnT[k][:, n0:n1],
                            start=(k == 0),
                            stop=(k == KB - 1),
                        )
                    nc.scalar.activation(
                        gatesT[:, n0:n1], ps[:, : n1 - n0],
                        mybir.ActivationFunctionType.Sigmoid,
                        bias=bg_t,
                    )

                # 1e: RoPE in place on qkT tiles.
                for i in range(16):
                    rot = p_rope.tile([P, T], bf16, tag="ropetmp")
                    for blk in range(4):
                        src = (blk // 2) * 64 + ((blk + 1) % 2) * 32
                        nc.vector.tensor_copy(
                            rot[blk * 32 : (blk + 1) * 32, :],
                            qkT[i][src : src + 32, :],
                        )
                    qc = p_rope.tile([P, T], bf16, tag="ropetmp", name="qc")
                    nc.vector.tensor_mul(qc, qkT[i], cos_t)
                    nc.vector.tensor_mul(rot, rot, sin_t)
                    nc.vector.tensor_add(qkT[i], qc, rot)

            # ---------------- Phase 2: attention ----------------------------
            with ExitStack() as ph2:
                p_y = ph2.enter_context(tc.tile_pool(name="p_y", bufs=1))
                y = [p_y.tile([P, NQ], bf16, tag=f"y{i}", name=f"y{i}") for i in range(KB)]
                p_wo = ph2.enter_context(tc.tile_pool(name="p_wo", bufs=1))
                wo_sb = [p_wo.tile([P, DIM], bf16, tag=f"wo{k}", name=f"wo{k}")
                         for k in range(KB)]
                for k in range(KB):
                    nc.gpsimd.dma_start(wo_sb[k], wo_d[k * P : (k + 1) * P, :])
                ph2ps = ph2.enter_context(ExitStack())
                p_expm = ph2ps.enter_context(tc.tile_pool(name="p_expm", bufs=9))
                p_nrm = ph2ps.enter_context(tc.tile_pool(name="p_nrm", bufs=2))
                p_pssim = ph2ps.enter_context(
                    tc.tile_pool(name="p_pssim", bufs=2, space="PSUM")
                )
                p_psav = ph2ps.enter_context(
                    tc.tile_pool(name="p_psav", bufs=1, space="PSUM")
                )

                def emit_qkexp(h):
                    qh = qkT[h // 2][64 * (h % 2) : 64 * (h % 2) + 64, :]
                    kh = qkT[8 + h // 2][64 * (h % 2) : 64 * (h % 2) + 64, :]
                    expm = []
                    for g in range(5):
                        js = range(4 * g, min(4 * g + 4, NT))
                        ps = p_pssim.tile([P, 1024], f32, tag="pssim", name="pssim")
                        ranges = []  # written column ranges (merged)
                        for j in js:
                            s = j - 4 * g
                            qs = P * max(j, 1)
                            qe = P * min(j + 2, NT)
                            nc.tensor.matmul(
                                ps[:, 256 * s : 256 * s + (qe - qs)],
                                kh[:, j * P : (j + 1) * P],
                                qh[:, qs:qe],
                                start=True,
                                stop=True,
                            )
                            c0, c1 = 256 * s, 256 * s + (qe - qs)
                            if ranges and ranges[-1][1] == c0:
                                ranges[-1][1] = c1
                            else:
                                ranges.append([c0, c1])
                        em = p_expm.tile([P, 1024], bf16, tag="expm", name="expm")
                        mk = mask0 if g == 0 else maskr
                        for c0, c1 in ranges:
                            nc.scalar.activation(
                                em[:, c0:c1], ps[:, c0:c1],
                                mybir.ActivationFunctionType.Exp,
                            )
                            nc.vector.tensor_mul(em[:, c0:c1], em[:, c0:c1],
                                                 mk[:, c0:c1])
                        expm.append(em)
                    return expm

                def emit_av_norm(h, expm):
                    # AV with overlapped accumulation; ones-column gives dens.
                    pav = [
                        p_psav.tile([65, 1024], f32, tag="pavlo", name="pavlo"),
                        p_psav.tile([65, 1024], f32, tag="pavhi", name="pavhi"),
                    ]
                    for j in range(NT):
                        g, s = j // 4, j % 4
                        qs = P * max(j, 1)
                        pieces = []
                        if j >= 1:
                            pieces.append((P * (j - 1), P * j, 256 * s + P * j - qs))
                        if j <= NT - 2:
                            pieces.append(
                                (P * j, P * (j + 1), 256 * s + P * (j + 1) - qs)
                            )
                        for (p0, p1, ec) in pieces:
                            hv = p0 // 1024
                            start = (p0 % 512 == 0) and (j == 4 * (p0 // 512))
                            stop = (p1 % 512 == 0) and (j == 4 * (p1 // 512))
                            nc.tensor.matmul(
                                pav[hv][:, p0 - 1024 * hv : p1 - 1024 * hv],
                                vsl[j][:, h * 65 : (h + 1) * 65],
                                expm[g][:, ec : ec + P],
                                start=start,
                                stop=stop,
                                skip_group_check=True,
                            )

                    # copy out of PSUM immediately (releases pav for the next
                    # head), then normalize + gate from SBUF.
                    for hv in range(2):
                        outun = p_nrm.tile([65, 1024], f32, tag="outun",
                                           name="outun", bufs=2)
                        nc.scalar.copy(outun, pav[hv])
                        den_sq = p_nrm.tile([8, 128], f32, tag="densq")
                        nc.sync.dma_start(den_sq, outun[64:65, :])
                        gate_sq = p_nrm.tile([8, 128], f32, tag="gatesq")
                        nc.sync.dma_start(
                            gate_sq,
                            gatesT[h : h + 1, P + 1024 * hv : P + 1024 * (hv + 1)],
                        )
                        rden = p_nrm.tile([8, 128], f32, tag="rden")
                        nc.vector.reciprocal(rden, den_sq)
                        scale_sq = p_nrm.tile([8, 128], bf16, tag="scalesq")
                        nc.vector.tensor_mul(scale_sq, rden, gate_sq)
                        nc.sync.dma_start(scr_d[h, hv, :], scale_sq)
                        scale_b = p_nrm.tile([64, 1024], bf16, tag="scaleb")
                        nc.sync.dma_start(
                            scale_b,
                            scr_d[h, hv, :].partition_broadcast(64),
                        )
                        nc.vector.tensor_mul(
                            y[h // 2][
                                64 * (h % 2) : 64 * (h % 2) + 64,
                                1024 * hv : 1024 * (hv + 1),
                            ],
                            outun[0:64, :],
                            scale_b,
                        )

                # software pipeline: QK/exp for head h+1 is emitted before
                # AV of head h so the in-order PE always has runnable work.
                expm_next = emit_qkexp(0)
                for h in range(H):
                    expm_cur = expm_next
                    if h + 1 < H:
                        expm_next = emit_qkexp(h + 1)
                    emit_av_norm(h, expm_cur)

                # ---------------- Phase 3: output projection -----------------
                ph2ps.close()  # release attention SBUF + PSUM pools
                with ExitStack() as ph3:
                    p_pso = ph3.enter_context(
                        tc.tile_pool(name="p_pso", bufs=4, space="PSUM")
                    )
                    p_out = ph3.enter_context(tc.tile_pool(name="p_out", bufs=3))
                    for mt in range(16):
                        o_t = p_out.tile([P, DIM], f32, tag="o")
                        for nh in range(2):
                            ps = p_pso.tile([P, 512], f32, tag="pso")
                            for k in range(KB):
                                nc.tensor.matmul(
                                    ps,
                                    y[k][:, mt * P : (mt + 1) * P],
                                    wo_sb[k][:, nh * 512 : (nh + 1) * 512],
                                    start=(k == 0),
                                    stop=(k == KB - 1),
                                )
                            nc.scalar.copy(o_t[:, nh * 512 : (nh + 1) * 512], ps)
                        nc.gpsimd.dma_start(out_d[mt * P : (mt + 1) * P, :], o_t)

    nc.finalize()
    return nc


def _get_program():
    if "nc" not in _PROGRAM_CACHE:
        _PROGRAM_CACHE["nc"] = _build_nc()
    return _PROGRAM_CACHE["nc"]


def _rope_tables(start: int):
    """cos/sinS tables [128, T] bf16 for shard starting at query index start.

    sinS carries the rotate-half sign so that
    rope(v) = v * cos + blockswap(v) * sinS.
    """
    pos = np.arange(start - WS, start + NQ, dtype=np.float64)
    np.clip(pos, 0, None, out=pos)
    inv = 10000.0 ** (-np.arange(0, DH, 2, dtype=np.float64) / DH)  # [32]
    d = np.arange(P) % DH
    theta = pos[None, :] * inv[d % 32][:, None]  # [128, T]
    cos = np.cos(theta)
    sinS = np.sin(theta) * np.where(d % DH < 32, -1.0, 1.0)[:, None]
    return cos.astype(BF16), sinS.astype(BF16)


def _masks(is_first_chunk: bool):
    k = np.arange(P)[:, None]
    i = np.arange(P)[None, :]
    own = (i >= k).astype(np.float32)        # queries in window j vs keys j
    look = (i <= k).astype(np.float32)       # queries in window j+1 vs keys j
    maskB = np.concatenate([own, look], axis=1)  # [128, 256]
    slot0 = np.concatenate(
        [np.zeros((P, P), np.float32) if is_first_chunk else look,
         np.zeros((P, P), np.float32)], axis=1)
    mask0 = np.concatenate([slot0] + [maskB] * 3, axis=1)
    maskr = np.concatenate([maskB] * 4, axis=1)
    return mask0.astype(BF16), maskr.astype(BF16)


def kernel(x, ln_w, ln_b, w_qkv, w_gate, b_gate, w_out):
    from concourse.bass_utils import run_bass_kernel_spmd

    x = np.asarray(x, dtype=np.float32)
    ln_w = np.asarray(ln_w, dtype=np.float32)
    ln_b = np.asarray(ln_b, dtype=np.float32)
    w_qkv = np.asarray(w_qkv, dtype=np.float32)
    w_gate = np.asarray(w_gate, dtype=np.float32)
    b_gate = np.asarray(b_gate, dtype=np.float32)
    w_out = np.asarray(w_out, dtype=np.float32)

    # Fold LayerNorm affine into the projections.
    wf = ln_w[:, None] * w_qkv                     # [DIM, 3*H*DH]
    bias_qkv = ln_b @ w_qkv                        # [3*H*DH]
    wgf = ln_w[:, None] * w_gate
    bgf = (b_gate + ln_b @ w_gate).astype(np.float32)

    # Fold the attention 1/sqrt(DH) scale into the q projection (RoPE is
    # linear, so pre-scaling q is equivalent).
    wqk_f = wf[:, : 2 * DIM].copy()
    wqk_f[:, :DIM] *= QSCALE
    bqk_f = bias_qkv[: 2 * DIM].astype(np.float32).copy()
    bqk_f[:DIM] *= QSCALE
    wqk = wqk_f.astype(BF16)
    wv = wf[:, 2 * DIM :].astype(BF16)
    bqk = bqk_f.reshape(2 * DIM, 1)
    bv = bias_qkv[2 * DIM :].astype(BF16).reshape(1, DIM)
    wg = wgf.astype(BF16)
    wo = w_out.astype(BF16)

    tabs = [_rope_tables(0), _rope_tables(NQ)]
    masks = [_masks(True), _masks(False)]

    in_maps = []
    for core in range(NCORES):
        b, half = core // 2, core % 2
        start = half * NQ
        if half == 0:
            x_sh = np.concatenate(
                [np.zeros((WS, DIM), np.float32), x[b, :NQ]], axis=0
            )
        else:
            x_sh = x[b, start - WS : start + NQ]
        cos, sin = tabs[half]
        mask0, maskr = masks[0 if half == 0 else 1]
        in_maps.append({
            "x": np.ascontiguousarray(x_sh),
            "wqk": wqk, "wv": wv, "wg": wg, "wo": wo,
            "bqk": bqk, "bv": bv, "bg": bgf.reshape(H, 1),
            "cos": cos, "sin": sin,
            "mask0": mask0, "maskr": maskr,
        })

    global _last_in_maps
    _last_in_maps = in_maps

    nc = _get_program()
    res = run_bass_kernel_spmd(nc, in_maps, list(range(NCORES)))

    out = np.empty((B, N, DIM), np.float32)
    for core in range(NCORES):
        b, half = core // 2, core % 2
        out[b, half * NQ : (half + 1) * NQ] = res.results[core]["out"]
    return out
